# revision 18
# baseline (speedup 1.0000x reference)
"""MoE layer (8 experts, top-2) on 8 Trainium2 NeuronCores.

Expert parallelism with host-side dispatch; fp8 DoubleRow matmuls with full
error compensation, all accumulating in a single fp32 PSUM group per output
tile:
  - Host: gate logits, top-2 + softmax, token->expert dispatch. Gate scales
    are folded into x (relu is positive-homogeneous), so the device computes
    plain y_e = relu(x_e @ w1.T) @ w2.T on pre-scaled tokens.
  - Every operand T is split T = T0 + T1 with both halves fp8 e4m3. The
    residual product rides in the same PSUM at matched scale by pre-scaling
    the weight residual up by 2^4 and the activation main down by 2^4 (both
    exact exponent shifts in fp8):
      psum = x0@w0.T + x1@w0.T + (x0/16)@((w-w0)*16).T
    dropping only the tiny residual*residual term. Each matmul pairs two
    128-deep k-tiles in MatmulPerfMode.DoubleRow (0.5 cycles/row = 4x bf16
    throughput), so the compensated total runs at 1.5x bf16 speed with
    bf16-level accuracy (~3e-3 end to end).
  - Layer 1: h0 = fp8(relu(psum)) (ACT), h1 = fp8(relu(psum)-h0) (DVE stt),
    h0d = h0/16 (ACT). Layer 2 repeats the same 3-group pattern on
    (h0, h1, h0d) against w2 splits, yT written back d-major.
  - Both layers keep tokens on the moving free dim: any block width, no
    128-token padding anywhere (capacity = max expert count, exactly).
  - DMA issue order is hand-scheduled: block0 x + first w1 chunks feed the
    first matmuls within ~2.5us; the w1 chunk stream stays ahead of L1; w2a
    then w2b follow so layer 2's mains/corrections are resident just in time.
"""

import os

os.environ.setdefault("BASS_NEVER_TRACE", "1")

import numpy as np
import ml_dtypes

D_MODEL = 1024
D_FF = 4096
NUM_EXPERTS = 8
TOP_K = 2
P = 128
KD = D_MODEL // P  # 8
KF = D_FF // P  # 32
C_BLK = 512
WCH = 4  # fc chunk size for w1 DMA staging
SC = 16.0  # residual pre-scale (2^4); shifted operands use 1/SC

F8 = ml_dtypes.float8_e4m3

_NC_CACHE: dict[int, object] = {}


def _block_widths(C: int) -> list[int]:
    """First block 512 (buys time for the w2 DMA stream before L2 starts),
    the rest equal-ish. Every block should be >=342 wide so the PE engine
    time per DoubleRow (0.5 cyc/row) stays above the 71 ns sequencer cost."""
    if C <= C_BLK:
        return [C]
    nb = -(-C // C_BLK)
    rest = C - C_BLK
    base = rest // (nb - 1)
    remn = rest % (nb - 1)
    widths = [C_BLK] + [base + (1 if i < remn else 0) for i in range(nb - 1)]
    assert sum(widths) == C
    return widths


def capacity(max_count: int) -> int:
    return max(max_count, 2 * 342)


def build_moe_nc(C: int):
    """Bass/Tile program for one expert shard with token capacity C.

    DRAM inputs (per core), all fp8 e4m3:
      xs0 [128, KD, C]        xs0[p,k,c] = fp8(g_c * x_c)[k*128+p]
      xs1 [128, KD, C]        fp8 residual of the above
      xsd [128, KD, C]        fp8(xs0 / 16) (exact shift)
      w1a [128, KF, KD, 128]  w1a[p,fc,k,j] = fp8(w1[fc*128+j, k*128+p])
      w1b [128, KF, KD, 128]  fp8((w1 - w1a)*16), same layout
      w2a [128, KF, D]        w2a[p,kf,d] = fp8(w2[d, kf*128+p])
      w2b [128, KF, D]        fp8((w2 - w2a)*16)
    DRAM output:
      yT  [128, KD, C] f32    yT[p,dt,c] = y[c, dt*128+p]
    """
    import concourse.mybir as mybir
    import concourse.tile as tile
    from concourse import bacc

    f8, f32 = mybir.dt.float8e4, mybir.dt.float32
    Relu = mybir.ActivationFunctionType.Relu
    DR = mybir.MatmulPerfMode.DoubleRow
    Alu = mybir.AluOpType

    widths = _block_widths(C)
    NB = len(widths)

    nc = bacc.Bacc("TRN2", target_bir_lowering=False, debug=False)
    # x inputs are laid out per 512-padded block so every DMA descriptor is a
    # full 512B contiguous run (keeps the DMA at full rate)
    xs0 = nc.dram_tensor("xs0", [P, KD, NB, C_BLK], f8, kind="ExternalInput")
    xs1 = nc.dram_tensor("xs1", [P, KD, NB, C_BLK], f8, kind="ExternalInput")
    w1a = nc.dram_tensor("w1a", [P, KF, KD, P], f8, kind="ExternalInput")
    w1b = nc.dram_tensor("w1b", [P, KF, KD, P], f8, kind="ExternalInput")
    w2a = nc.dram_tensor("w2a", [P, KF, D_MODEL], f8, kind="ExternalInput")
    w2b = nc.dram_tensor("w2b", [P, KF, D_MODEL], f8, kind="ExternalInput")
    yT = nc.dram_tensor("yT", [P, KD, C], f32, kind="ExternalOutput")

    blocks = []
    off = 0
    for w in widths:
        blocks.append((off, w))
        off += w

    with tile.TileContext(nc) as tc:
        with (
            tc.tile_pool(name="wpool", bufs=1) as wpool,
            tc.tile_pool(name="xpool", bufs=2) as xpool,
            tc.tile_pool(name="hpool", bufs=1) as hpool,
            tc.tile_pool(name="ypool", bufs=3) as ypool,
            tc.tile_pool(name="pmp", bufs=4, space="PSUM") as pmp,
            tc.tile_pool(name="pymp", bufs=3, space="PSUM") as pymp,
        ):
            # ---- SBUF weight tiles (resident) ----
            w1a_ch = [
                wpool.tile([P, WCH, KD, P], f8, tag=f"w1a_{c0}", name=f"w1a_{c0}")
                for c0 in range(0, KF, WCH)
            ]
            w1b_ch = [
                wpool.tile([P, WCH, KD, P], f8, tag=f"w1b_{c0}", name=f"w1b_{c0}")
                for c0 in range(0, KF, WCH)
            ]
            w2a_t = wpool.tile([P, KF, D_MODEL], f8, tag="w2a", name="w2a_t")
            w2b_t = wpool.tile([P, KF, D_MODEL], f8, tag="w2b", name="w2b_t")

            # ---- block 0 x tiles + head-of-stream DMAs (criticality order:
            # each piece lands just before the matmul group that reads it) ----
            xt0 = xpool.tile([P, KD, C_BLK], f8, tag="xt0", name="xt0_0")
            xt1 = xpool.tile([P, KD, C_BLK], f8, tag="xt1", name="xt1_0")
            xtd = xpool.tile([P, KD, C_BLK], f8, tag="xtd", name="xtd_0")
            # interleave the first f-tile's weight k-pairs with x k-pairs so the
            # very first matmul gates on only ~64KB of DMA
            for k0 in range(0, KD, 2):
                nc.sync.dma_start(w1a_ch[0][:, 0:1, k0 : k0 + 2], w1a[:, 0:1, k0 : k0 + 2])
                nc.sync.dma_start(xt0[:, k0 : k0 + 2], xs0[:, k0 : k0 + 2, 0])
            for k0 in range(0, KD, 2):
                nc.sync.dma_start(xt1[:, k0 : k0 + 2], xs1[:, k0 : k0 + 2, 0])
            for k0 in range(0, KD, 2):
                nc.vector.tensor_scalar_mul(
                    xtd[:, k0 : k0 + 2], xt0[:, k0 : k0 + 2], 1.0 / SC
                )
            nc.sync.dma_start(w1b_ch[0][:, 0:1], w1b[:, 0:1])
            for j in range(1, WCH):
                nc.sync.dma_start(w1a_ch[0][:, j : j + 1], w1a[:, j : j + 1])
                nc.sync.dma_start(w1b_ch[0][:, j : j + 1], w1b[:, j : j + 1])
            # w1 chunk stream (stays well ahead of L1 consumption), then w2a
            # halves (layer-2 mains), then w2b halves (layer-2 corrections)
            for c0 in range(WCH, KF, WCH):
                ci = c0 // WCH
                nc.sync.dma_start(w1a_ch[ci][:], w1a[:, c0 : c0 + WCH])
                nc.sync.dma_start(w1b_ch[ci][:], w1b[:, c0 : c0 + WCH])
            DH = D_MODEL // 2
            for d0 in (0, DH):
                nc.sync.dma_start(w2a_t[:, :, d0 : d0 + DH], w2a[:, :, d0 : d0 + DH])
            for d0 in (0, DH):
                nc.sync.dma_start(w2b_t[:, :, d0 : d0 + DH], w2b[:, :, d0 : d0 + DH])

            def w1a_ap(fc, kp):
                return w1a_ch[fc // WCH][:, fc % WCH, 2 * kp : 2 * kp + 2, :]

            def w1b_ap(fc, kp):
                return w1b_ch[fc // WCH][:, fc % WCH, 2 * kp : 2 * kp + 2, :]

            xts = {0: (xt0, xt1, xtd)}
            for bi, (off, w) in enumerate(blocks):
                xt0, xt1, xtd = xts.pop(bi)
                h0 = hpool.tile([P, KF, C_BLK], f8, tag="h0", name=f"h0_{bi}")
                h1 = hpool.tile([P, KF, C_BLK], f8, tag="h1", name=f"h1_{bi}")
                h0d = hpool.tile([P, KF, C_BLK], f8, tag="h0d", name=f"h0d_{bi}")
                # ---- layer 1: 12 DoubleRows into one PSUM per f-tile ----
                for fc in range(KF):
                    pm = pmp.tile([P, C_BLK], f32, tag="pm", name=f"pm_{bi}_{fc}")
                    for kp in range(KD // 2):
                        nc.tensor.matmul(
                            pm[:, :w], lhsT=w1a_ap(fc, kp),
                            rhs=xt0[:, 2 * kp : 2 * kp + 2, :w],
                            start=(kp == 0), stop=False, perf_mode=DR,
                        )
                    for kp in range(KD // 2):
                        nc.tensor.matmul(
                            pm[:, :w], lhsT=w1a_ap(fc, kp),
                            rhs=xt1[:, 2 * kp : 2 * kp + 2, :w],
                            start=False, stop=False, perf_mode=DR,
                        )
                    for kp in range(KD // 2):
                        nc.tensor.matmul(
                            pm[:, :w], lhsT=w1b_ap(fc, kp),
                            rhs=xtd[:, 2 * kp : 2 * kp + 2, :w],
                            start=False, stop=(kp == KD // 2 - 1), perf_mode=DR,
                        )
                    nc.scalar.activation(h0[:, fc, :w], pm[:, :w], Relu)
                    nc.vector.scalar_tensor_tensor(
                        h1[:, fc, :w], pm[:, :w], 0.0, h0[:, fc, :w],
                        Alu.max, Alu.subtract,
                    )
                    nc.scalar.mul(h0d[:, fc, :w], h0[:, fc, :w], 1.0 / SC)
                # prefetch next block's x before layer 2's y DMAs hit the queue
                if bi + 1 < len(blocks):
                    nxt0 = xpool.tile([P, KD, C_BLK], f8, tag="xt0", name=f"xt0_{bi+1}")
                    nxt1 = xpool.tile([P, KD, C_BLK], f8, tag="xt1", name=f"xt1_{bi+1}")
                    nxtd = xpool.tile([P, KD, C_BLK], f8, tag="xtd", name=f"xtd_{bi+1}")
                    nc.sync.dma_start(nxt0[:], xs0[:, :, bi + 1])
                    nc.sync.dma_start(nxt1[:], xs1[:, :, bi + 1])
                    for k0 in range(0, KD, 2):
                        nc.vector.tensor_scalar_mul(
                            nxtd[:, k0 : k0 + 2], nxt0[:, k0 : k0 + 2], 1.0 / SC
                        )
                    xts[bi + 1] = (nxt0, nxt1, nxtd)
                # ---- layer 2: 48 DoubleRows into one PSUM per d-tile ----
                for dt in range(KD):
                    pym = pymp.tile([P, C_BLK], f32, tag="pym", name=f"pym_{bi}_{dt}")
                    dsl = slice(dt * P, (dt + 1) * P)
                    for kp in range(KF // 2):
                        nc.tensor.matmul(
                            pym[:, :w], lhsT=w2a_t[:, 2 * kp : 2 * kp + 2, dsl],
                            rhs=h0[:, 2 * kp : 2 * kp + 2, :w],
                            start=(kp == 0), stop=False, perf_mode=DR,
                        )
                    for kp in range(KF // 2):
                        nc.tensor.matmul(
                            pym[:, :w], lhsT=w2a_t[:, 2 * kp : 2 * kp + 2, dsl],
                            rhs=h1[:, 2 * kp : 2 * kp + 2, :w],
                            start=False, stop=False, perf_mode=DR,
                        )
                    for kp in range(KF // 2):
                        nc.tensor.matmul(
                            pym[:, :w], lhsT=w2b_t[:, 2 * kp : 2 * kp + 2, dsl],
                            rhs=h0d[:, 2 * kp : 2 * kp + 2, :w],
                            start=False, stop=(kp == KF // 2 - 1), perf_mode=DR,
                        )
                    yt = ypool.tile([P, C_BLK], f32, tag="yt", name=f"yt_{bi}_{dt}")
                    nc.scalar.copy(yt[:, :w], pym[:, :w])
                    if bi == len(blocks) - 1 and dt == KD - 1:
                        # two half DMAs on separate queues shorten the drain
                        h = w // 2
                        nc.sync.dma_start(yT[:, dt, off : off + h], yt[:, :h])
                        nc.sync.dma_start(yT[:, dt, off + h : off + w], yt[:, h:w])
                    else:
                        nc.sync.dma_start(yT[:, dt, off : off + w], yt[:, :w])

    nc.compile()
    return nc


def route_tokens(xf: np.ndarray, gate_w: np.ndarray):
    """Top-2 routing, replicating jax.lax.top_k tie-breaking (lowest index)."""
    logits = xf @ gate_w.astype(np.float32).T  # [T, E]
    top2 = np.argsort(-logits, axis=-1, kind="stable")[:, :TOP_K]
    tv = np.take_along_axis(logits, top2, axis=-1)
    tv = tv - tv.max(axis=-1, keepdims=True)
    ex = np.exp(tv)
    gates = ex / ex.sum(axis=-1, keepdims=True)
    rows, weights = [], []
    for e in range(NUM_EXPERTS):
        r, kpos = np.nonzero(top2 == e)
        rows.append(r)
        weights.append(gates[r, kpos].astype(np.float32))
    return rows, weights


def _fp8_pair(a: np.ndarray, scale: float = 1.0):
    """a ~= a0 + a1/scale with both halves fp8 e4m3."""
    a0 = a.astype(F8)
    a1 = ((a - a0.astype(np.float32)) * scale).astype(F8)
    return a0, a1


def make_expert_inputs(xf, w1, w2, rows, weights, C):
    """Per-core input arrays in the DRAM layouts build_moe_nc expects."""
    widths = _block_widths(C)
    NB = len(widths)
    starts = np.cumsum([0] + widths[:-1])
    in_maps = []
    for e in range(NUM_EXPERTS):
        cnt = len(rows[e])
        Xg = np.zeros((C, D_MODEL), np.float32)
        Xg[:cnt] = xf[rows[e]] * weights[e][:, None]
        X0, X1 = _fp8_pair(Xg)

        def xlay(X):
            # [P, KD, C] -> 512-padded blocks [P, KD, NB, 512]
            xt = X.T.reshape(KD, P, C).transpose(1, 0, 2)  # [P, KD, C]
            out = np.zeros((P, KD, NB, C_BLK), X.dtype)
            for bi, (o, w) in enumerate(zip(starts, widths)):
                out[:, :, bi, :w] = xt[:, :, o : o + w]
            return out

        W1_0, W1_1 = _fp8_pair(w1[e].astype(np.float32), SC)

        def w1lay(W):
            return np.ascontiguousarray(W.reshape(KF, P, KD, P).transpose(3, 0, 2, 1))

        W2_0, W2_1 = _fp8_pair(w2[e].astype(np.float32), SC)

        def w2lay(W):
            return np.ascontiguousarray(W.T.reshape(KF, P, D_MODEL).transpose(1, 0, 2))

        in_maps.append(
            {
                "xs0": xlay(X0),
                "xs1": xlay(X1),
                "w1a": w1lay(W1_0),
                "w1b": w1lay(W1_1),
                "w2a": w2lay(W2_0),
                "w2b": w2lay(W2_1),
            }
        )
    return in_maps


def kernel(x, gate_w, w1, w2):
    from concourse.bass_utils import run_bass_kernel_spmd

    x = np.asarray(x)
    gate_w = np.asarray(gate_w)
    w1 = np.asarray(w1)
    w2 = np.asarray(w2)
    B, S, D = x.shape

    xf = x.reshape(-1, D).astype(np.float32)
    rows, weights = route_tokens(xf, gate_w)
    counts = [len(r) for r in rows]
    C = capacity(max(counts))

    nc = _NC_CACHE.get(C)
    if nc is None:
        nc = _NC_CACHE[C] = build_moe_nc(C)
    in_maps = make_expert_inputs(xf, w1, w2, rows, weights, C)
    res = run_bass_kernel_spmd(nc, in_maps, core_ids=list(range(NUM_EXPERTS)))

    out = np.zeros((B * S, D), np.float32)
    for e in range(NUM_EXPERTS):
        yT = res.results[e]["yT"]  # [P, KD, C]
        y = yT.transpose(2, 1, 0).reshape(C, D_MODEL)
        out[rows[e]] += y[: counts[e]]
    return out.reshape(B, S, D)


# revision 22
# speedup vs baseline: 1.0246x; 1.0246x over previous
"""MoE layer (8 experts, top-2) on 8 Trainium2 NeuronCores — balanced F-slice
variant.

Same fp8 DoubleRow + error-compensation scheme as the expert-parallel kernel,
but each core processes ALL 16384 (token, expert) assignments restricted to a
512-wide slice of the FFN hidden dim (core i owns hidden columns
[512*i, 512*(i+1)) of every expert). relu acts elementwise on h, so F-slices
are independent; each core emits a partial y (sum over its slice) in bf16 and
the host adds the 8 partials during scatter-add. Per-core compute is exactly
total/8 — perfect load balance with zero capacity padding — and the segment
schedule (expert order + lengths) is identical on every core, so one SPMD
program serves all.

Engine budget per 512-token block (~10.2us of PE): ACT does h0 relu-quant,
h0d shift, and half the y copies; DVE does h1 residuals and the xtd shift;
Pool (GPSIMD) does the other half of the y copies.
"""

import os

os.environ.setdefault("BASS_NEVER_TRACE", "1")

import numpy as np
import ml_dtypes

D_MODEL = 1024
D_FF = 4096
NUM_EXPERTS = 8
TOP_K = 2
P = 128
KD = D_MODEL // P  # 8
KS = 4  # k-tiles in a core's 512-wide F slice
FS = KS * P  # 512 hidden columns per core
C_BLK = 512
SC = 16.0

F8 = ml_dtypes.float8_e4m3
BF16 = ml_dtypes.bfloat16

_NC_CACHE: dict[tuple, object] = {}


def _seg_blocks(seg_lens):
    """Flat block schedule [(seg, off_global, width, block_index)]."""
    sched = []
    off = 0
    bidx = 0
    for s, L in enumerate(seg_lens):
        nb = max(1, -(-L // C_BLK))
        base = L // nb
        remn = L % nb
        for i in range(nb):
            w = base + (1 if i < remn else 0)
            sched.append((s, off, w, bidx))
            off += w
            bidx += 1
    return sched


def build_moe_nc2(seg_lens):
    """Bass/Tile program; seg_lens = per-expert assignment counts (same order
    on every core).

    DRAM inputs (per core), fp8 e4m3 unless noted:
      xs0 [P, KD, NB, 512]    gate-scaled token stream, expert-sorted, blocked
      xs1 [P, KD, NB, 512]    fp8 residual
      w1a [P, 8, KS, KD, P]   w1a[p,s,fc,k,j] = fp8(w1[s][slice_i*512+fc*128+j,
                              k*128+p]);  w1b = fp8((.)-w1a)*16
      w2a [P, 8, KS, D]       w2a[p,s,kf,d] = fp8(w2[s][d, slice_i*512+kf*128+p])
                              w2b = fp8 residual * 16
    DRAM output:
      yT [P, KD, C] bf16      partial y over this core's F slice, d-major
    """
    import concourse.mybir as mybir
    import concourse.tile as tile
    from concourse import bacc

    f8, f32, bf16 = mybir.dt.float8e4, mybir.dt.float32, mybir.dt.bfloat16
    Relu = mybir.ActivationFunctionType.Relu
    DR = mybir.MatmulPerfMode.DoubleRow
    Alu = mybir.AluOpType

    sched = _seg_blocks(seg_lens)
    NB = len(sched)
    C = sum(seg_lens)

    nc = bacc.Bacc("TRN2", target_bir_lowering=False, debug=False)
    xs0 = nc.dram_tensor("xs0", [P, KD, NB, C_BLK], f8, kind="ExternalInput")
    xs1 = nc.dram_tensor("xs1", [P, KD, NB, C_BLK], f8, kind="ExternalInput")
    w1a = nc.dram_tensor("w1a", [P, NUM_EXPERTS, KS, KD, P], f8, kind="ExternalInput")
    w1b = nc.dram_tensor("w1b", [P, NUM_EXPERTS, KS, KD, P], f8, kind="ExternalInput")
    w2a = nc.dram_tensor("w2a", [P, NUM_EXPERTS, KS, D_MODEL], f8, kind="ExternalInput")
    w2b = nc.dram_tensor("w2b", [P, NUM_EXPERTS, KS, D_MODEL], f8, kind="ExternalInput")
    yT = nc.dram_tensor("yT", [P, KD, C], bf16, kind="ExternalOutput")

    with tile.TileContext(nc) as tc:
        with (
            tc.tile_pool(name="wpool", bufs=1) as wpool,
            tc.tile_pool(name="xpool", bufs=4) as xpool,
            tc.tile_pool(name="hpool", bufs=2) as hpool,
            tc.tile_pool(name="ypool", bufs=8) as ypool,
            tc.tile_pool(name="pmp", bufs=4, space="PSUM") as pmp,
            tc.tile_pool(name="pymp", bufs=4, space="PSUM") as pymp,
        ):
            w1a_t = wpool.tile([P, NUM_EXPERTS, KS, KD, P], f8, tag="w1a", name="w1a_t")
            w1b_t = wpool.tile([P, NUM_EXPERTS, KS, KD, P], f8, tag="w1b", name="w1b_t")
            w2a_t = wpool.tile([P, NUM_EXPERTS, KS, D_MODEL], f8, tag="w2a", name="w2a_t")
            w2b_t = wpool.tile([P, NUM_EXPERTS, KS, D_MODEL], f8, tag="w2b", name="w2b_t")

            # head of DMA stream: segment-0 L1 weights + first x block, pieced
            # so the first matmuls start ASAP; then the rest of the weights
            xt0 = xpool.tile([P, KD, C_BLK], f8, tag="xt0", name="xt0_0")
            xt1 = xpool.tile([P, KD, C_BLK], f8, tag="xt1", name="xt1_0")
            xtd = xpool.tile([P, KD, C_BLK], f8, tag="xtd", name="xtd_0")
            nc.sync.dma_start(w1a_t[:, 0, 0:1], w1a[:, 0, 0:1])
            for k0 in range(0, KD, 2):
                nc.sync.dma_start(xt0[:, k0 : k0 + 2], xs0[:, k0 : k0 + 2, 0])
            for k0 in range(0, KD, 2):
                nc.sync.dma_start(xt1[:, k0 : k0 + 2], xs1[:, k0 : k0 + 2, 0])
            for k0 in range(0, KD, 2):
                eng = nc.vector
                eng.tensor_scalar_mul(
                    xtd[:, k0 : k0 + 2], xt0[:, k0 : k0 + 2], 1.0 / SC
                )
            nc.sync.dma_start(w1b_t[:, 0, 0:1], w1b[:, 0, 0:1])
            for j in range(1, KS):
                nc.sync.dma_start(w1a_t[:, 0, j : j + 1], w1a[:, 0, j : j + 1])
                nc.sync.dma_start(w1b_t[:, 0, j : j + 1], w1b[:, 0, j : j + 1])
            nc.sync.dma_start(w2a_t[:, 0], w2a[:, 0])
            nc.sync.dma_start(w2b_t[:, 0], w2b[:, 0])
            nc.sync.dma_start(w1a_t[:, 1], w1a[:, 1])
            nc.sync.dma_start(w1b_t[:, 1], w1b[:, 1])

            xts = {0: (xt0, xt1, xtd)}
            for s, off, w, bidx in sched:
                # trickle weights one segment ahead of use: at segment s's
                # first block, queue seg s+1's layer-2 and seg s+2's layer-1
                if bidx == 1:
                    nc.sync.dma_start(w2a_t[:, 1], w2a[:, 1])
                    nc.sync.dma_start(w2b_t[:, 1], w2b[:, 1])
                    nc.sync.dma_start(w1a_t[:, 2], w1a[:, 2])
                    nc.sync.dma_start(w1b_t[:, 2], w1b[:, 2])
                if bidx > 0 and sched[bidx - 1][0] == s - 1:
                    if s + 1 < NUM_EXPERTS:
                        nc.sync.dma_start(w2a_t[:, s + 1], w2a[:, s + 1])
                        nc.sync.dma_start(w2b_t[:, s + 1], w2b[:, s + 1])
                    if s + 2 < NUM_EXPERTS:
                        nc.sync.dma_start(w1a_t[:, s + 2], w1a[:, s + 2])
                        nc.sync.dma_start(w1b_t[:, s + 2], w1b[:, s + 2])
                xt0, xt1, xtd = xts.pop(bidx)
                h0 = hpool.tile([P, KS, C_BLK], f8, tag="h0", name=f"h0_{bidx}")
                h1 = hpool.tile([P, KS, C_BLK], f8, tag="h1", name=f"h1_{bidx}")
                h0d = hpool.tile([P, KS, C_BLK], f8, tag="h0d", name=f"h0d_{bidx}")
                # ---- layer 1 (12 DoubleRows per f-tile, one PSUM group) ----
                for fc in range(KS):
                    pm = pmp.tile([P, C_BLK], f32, tag="pm", name=f"pm_{bidx}_{fc}")
                    for kp in range(KD // 2):
                        nc.tensor.matmul(
                            pm[:, :w], lhsT=w1a_t[:, s, fc, 2 * kp : 2 * kp + 2],
                            rhs=xt0[:, 2 * kp : 2 * kp + 2, :w],
                            start=(kp == 0), stop=False, perf_mode=DR,
                        )
                    for kp in range(KD // 2):
                        nc.tensor.matmul(
                            pm[:, :w], lhsT=w1a_t[:, s, fc, 2 * kp : 2 * kp + 2],
                            rhs=xt1[:, 2 * kp : 2 * kp + 2, :w],
                            start=False, stop=False, perf_mode=DR,
                        )
                    for kp in range(KD // 2):
                        nc.tensor.matmul(
                            pm[:, :w], lhsT=w1b_t[:, s, fc, 2 * kp : 2 * kp + 2],
                            rhs=xtd[:, 2 * kp : 2 * kp + 2, :w],
                            start=False, stop=(kp == KD // 2 - 1), perf_mode=DR,
                        )
                    nc.scalar.activation(h0[:, fc, :w], pm[:, :w], Relu)
                    nc.vector.scalar_tensor_tensor(
                        h1[:, fc, :w], pm[:, :w], 0.0, h0[:, fc, :w],
                        Alu.max, Alu.subtract,
                    )
                    nc.gpsimd.tensor_scalar_mul(h0d[:, fc, :w], h0[:, fc, :w], 1.0 / SC)
                # prefetch next block's x
                if bidx + 1 < NB:
                    nxt0 = xpool.tile([P, KD, C_BLK], f8, tag="xt0", name=f"xt0_{bidx+1}")
                    nxt1 = xpool.tile([P, KD, C_BLK], f8, tag="xt1", name=f"xt1_{bidx+1}")
                    nxtd = xpool.tile([P, KD, C_BLK], f8, tag="xtd", name=f"xtd_{bidx+1}")
                    nc.sync.dma_start(nxt0[:], xs0[:, :, bidx + 1])
                    nc.sync.dma_start(nxt1[:], xs1[:, :, bidx + 1])
                    for k0 in range(0, KD, 2):
                        eng = nc.vector
                        eng.tensor_scalar_mul(
                            nxtd[:, k0 : k0 + 2], nxt0[:, k0 : k0 + 2], 1.0 / SC
                        )
                    xts[bidx + 1] = (nxt0, nxt1, nxtd)
                # ---- layer 2 (6 DoubleRows per d-tile, one PSUM group) ----
                for dt in range(KD):
                    pym = pymp.tile([P, C_BLK], f32, tag="pym", name=f"pym_{bidx}_{dt}")
                    dsl = slice(dt * P, (dt + 1) * P)
                    for kp in range(KS // 2):
                        nc.tensor.matmul(
                            pym[:, :w], lhsT=w2a_t[:, s, 2 * kp : 2 * kp + 2, dsl],
                            rhs=h0[:, 2 * kp : 2 * kp + 2, :w],
                            start=(kp == 0), stop=False, perf_mode=DR,
                        )
                    for kp in range(KS // 2):
                        nc.tensor.matmul(
                            pym[:, :w], lhsT=w2a_t[:, s, 2 * kp : 2 * kp + 2, dsl],
                            rhs=h1[:, 2 * kp : 2 * kp + 2, :w],
                            start=False, stop=False, perf_mode=DR,
                        )
                    for kp in range(KS // 2):
                        nc.tensor.matmul(
                            pym[:, :w], lhsT=w2b_t[:, s, 2 * kp : 2 * kp + 2, dsl],
                            rhs=h0d[:, 2 * kp : 2 * kp + 2, :w],
                            start=False, stop=(kp == KS // 2 - 1), perf_mode=DR,
                        )
                    yt = ypool.tile([P, C_BLK], bf16, tag="yt", name=f"yt_{bidx}_{dt}")
                    nc.scalar.copy(yt[:, :w], pym[:, :w])
                    nc.sync.dma_start(yT[:, dt, off : off + w], yt[:, :w])

    nc.compile()
    return nc


def route_tokens(xf: np.ndarray, gate_w: np.ndarray):
    logits = xf @ gate_w.astype(np.float32).T
    top2 = np.argsort(-logits, axis=-1, kind="stable")[:, :TOP_K]
    tv = np.take_along_axis(logits, top2, axis=-1)
    tv = tv - tv.max(axis=-1, keepdims=True)
    ex = np.exp(tv)
    gates = ex / ex.sum(axis=-1, keepdims=True)
    rows, weights = [], []
    for e in range(NUM_EXPERTS):
        r, kpos = np.nonzero(top2 == e)
        rows.append(r)
        weights.append(gates[r, kpos].astype(np.float32))
    return rows, weights


def _fp8_pair(a: np.ndarray, scale: float = 1.0):
    a0 = a.astype(F8)
    a1 = ((a - a0.astype(np.float32)) * scale).astype(F8)
    return a0, a1


def make_inputs(xf, w1, w2, rows, weights):
    seg_lens = tuple(len(r) for r in rows)
    C = sum(seg_lens)
    sched = _seg_blocks(seg_lens)
    NB = len(sched)

    Xg = np.empty((C, D_MODEL), np.float32)
    o = 0
    for e in range(NUM_EXPERTS):
        n = len(rows[e])
        Xg[o : o + n] = xf[rows[e]] * weights[e][:, None]
        o += n
    X0, X1 = _fp8_pair(Xg)

    def xlay(X):
        xt = X.T.reshape(KD, P, C).transpose(1, 0, 2)  # [P, KD, C]
        out = np.zeros((P, KD, NB, C_BLK), X.dtype)
        for s, off, w, bidx in sched:
            out[:, :, bidx, :w] = xt[:, :, off : off + w]
        return out

    xs0, xs1 = xlay(X0), xlay(X1)

    # full-expert weight layouts, then slice per core
    w1a_full, w1b_full, w2a_full, w2b_full = [], [], [], []
    for e in range(NUM_EXPERTS):
        A, B = _fp8_pair(w1[e].astype(np.float32), SC)
        w1a_full.append(A.reshape(KF := D_FF // P, P, KD, P).transpose(3, 0, 2, 1))
        w1b_full.append(B.reshape(KF, P, KD, P).transpose(3, 0, 2, 1))
        A2, B2 = _fp8_pair(w2[e].astype(np.float32), SC)
        w2a_full.append(A2.T.reshape(KF, P, D_MODEL).transpose(1, 0, 2))
        w2b_full.append(B2.T.reshape(KF, P, D_MODEL).transpose(1, 0, 2))

    in_maps = []
    for i in range(NUM_EXPERTS):  # core i -> F slice i
        sl = slice(KS * i, KS * (i + 1))
        in_maps.append(
            {
                "xs0": xs0,
                "xs1": xs1,
                "w1a": np.ascontiguousarray(
                    np.stack([w1a_full[e][:, sl] for e in range(NUM_EXPERTS)], axis=1)
                ),
                "w1b": np.ascontiguousarray(
                    np.stack([w1b_full[e][:, sl] for e in range(NUM_EXPERTS)], axis=1)
                ),
                "w2a": np.ascontiguousarray(
                    np.stack([w2a_full[e][:, sl] for e in range(NUM_EXPERTS)], axis=1)
                ),
                "w2b": np.ascontiguousarray(
                    np.stack([w2b_full[e][:, sl] for e in range(NUM_EXPERTS)], axis=1)
                ),
            }
        )
    return in_maps, seg_lens


def kernel(x, gate_w, w1, w2):
    from concourse.bass_utils import run_bass_kernel_spmd

    x = np.asarray(x)
    B, S, D = x.shape
    xf = x.reshape(-1, D).astype(np.float32)
    rows, weights = route_tokens(xf, np.asarray(gate_w))
    in_maps, seg_lens = make_inputs(xf, np.asarray(w1), np.asarray(w2), rows, weights)

    nc = _NC_CACHE.get(seg_lens)
    if nc is None:
        nc = _NC_CACHE[seg_lens] = build_moe_nc2(seg_lens)
    res = run_bass_kernel_spmd(nc, in_maps, core_ids=list(range(NUM_EXPERTS)))

    C = sum(seg_lens)
    ysum = np.zeros((C, D_MODEL), np.float32)
    for c in range(NUM_EXPERTS):
        yT = res.results[c]["yT"].astype(np.float32)  # [P, KD, C]
        ysum += yT.transpose(2, 1, 0).reshape(C, D_MODEL)
    out = np.zeros((B * S, D), np.float32)
    o = 0
    for e in range(NUM_EXPERTS):
        n = len(rows[e])
        out[rows[e]] += ysum[o : o + n]
        o += n
    return out.reshape(B, S, D)


# revision 28
# speedup vs baseline: 1.0658x; 1.0402x over previous
"""MoE layer (8 experts, top-2) on 8 Trainium2 NeuronCores — balanced F-slice
variant.

Same fp8 DoubleRow + error-compensation scheme as the expert-parallel kernel,
but each core processes ALL 16384 (token, expert) assignments restricted to a
512-wide slice of the FFN hidden dim (core i owns hidden columns
[512*i, 512*(i+1)) of every expert). relu acts elementwise on h, so F-slices
are independent; each core emits a partial y (sum over its slice) in bf16 and
the host adds the 8 partials during scatter-add. Per-core compute is exactly
total/8 — perfect load balance with zero capacity padding — and the segment
schedule (expert order + lengths) is identical on every core, so one SPMD
program serves all.

Engine budget per 512-token block (~10.2us of PE): ACT does h0 relu-quant,
h0d shift, and half the y copies; DVE does h1 residuals and the xtd shift;
Pool (GPSIMD) does the other half of the y copies.
"""

import os

os.environ.setdefault("BASS_NEVER_TRACE", "1")

import numpy as np
import ml_dtypes

D_MODEL = 1024
D_FF = 4096
NUM_EXPERTS = 8
TOP_K = 2
P = 128
KD = D_MODEL // P  # 8
KS = 4  # k-tiles in a core's 512-wide F slice
FS = KS * P  # 512 hidden columns per core
C_BLK = 512
SC = 16.0

F8 = ml_dtypes.float8_e4m3
BF16 = ml_dtypes.bfloat16

_NC_CACHE: dict[tuple, object] = {}


def _seg_blocks(seg_lens):
    """Flat block schedule [(seg, off_global, width, block_index)]."""
    sched = []
    off = 0
    bidx = 0
    for s, L in enumerate(seg_lens):
        nb = max(1, -(-L // C_BLK))
        base = L // nb
        remn = L % nb
        for i in range(nb):
            w = base + (1 if i < remn else 0)
            sched.append((s, off, w, bidx))
            off += w
            bidx += 1
    return sched


def build_moe_nc2(seg_lens):
    """Bass/Tile program; seg_lens = per-expert assignment counts (same order
    on every core).

    DRAM inputs (per core), fp8 e4m3 unless noted:
      xs0 [P, KD, NB, 512]    gate-scaled token stream, expert-sorted, blocked
      xs1 [P, KD, NB, 512]    fp8 residual
      w1a [P, 8, KS, KD, P]   w1a[p,s,fc,k,j] = fp8(w1[s][slice_i*512+fc*128+j,
                              k*128+p]);  w1b = fp8((.)-w1a)*16
      w2a [P, 8, KS, D]       w2a[p,s,kf,d] = fp8(w2[s][d, slice_i*512+kf*128+p])
                              w2b = fp8 residual * 16
    DRAM output:
      yT [P, KD, C] bf16      partial y over this core's F slice, d-major
    """
    import concourse.mybir as mybir
    import concourse.tile as tile
    from concourse import bacc

    f8, f32, bf16 = mybir.dt.float8e4, mybir.dt.float32, mybir.dt.bfloat16
    Relu = mybir.ActivationFunctionType.Relu
    DR = mybir.MatmulPerfMode.DoubleRow
    Alu = mybir.AluOpType

    sched = _seg_blocks(seg_lens)
    NB = len(sched)
    C = sum(seg_lens)

    nc = bacc.Bacc("TRN2", target_bir_lowering=False, debug=False)
    xs0 = nc.dram_tensor("xs0", [P, KD, NB, C_BLK], f8, kind="ExternalInput")
    xs1 = nc.dram_tensor("xs1", [P, KD, NB, C_BLK], f8, kind="ExternalInput")
    w1a = nc.dram_tensor("w1a", [P, NUM_EXPERTS, KS, KD, P], f8, kind="ExternalInput")
    w1b = nc.dram_tensor("w1b", [P, NUM_EXPERTS, KS, KD, P], f8, kind="ExternalInput")
    w2a = nc.dram_tensor("w2a", [P, NUM_EXPERTS, KS, D_MODEL], f8, kind="ExternalInput")
    w2b = nc.dram_tensor("w2b", [P, NUM_EXPERTS, KS, D_MODEL], f8, kind="ExternalInput")
    yT = nc.dram_tensor("yT", [P, KD, C], bf16, kind="ExternalOutput")

    with tile.TileContext(nc) as tc:
        with (
            tc.tile_pool(name="wpool", bufs=1) as wpool,
            tc.tile_pool(name="xpool", bufs=4) as xpool,
            tc.tile_pool(name="hpool", bufs=2) as hpool,
            tc.tile_pool(name="ypool", bufs=8) as ypool,
            tc.tile_pool(name="pmp", bufs=4, space="PSUM") as pmp,
            tc.tile_pool(name="pymp", bufs=4, space="PSUM") as pymp,
        ):
            w1a_t = wpool.tile([P, NUM_EXPERTS, KS, KD, P], f8, tag="w1a", name="w1a_t")
            w1b_t = wpool.tile([P, NUM_EXPERTS, KS, KD, P], f8, tag="w1b", name="w1b_t")
            w2a_t = wpool.tile([P, NUM_EXPERTS, KS, D_MODEL], f8, tag="w2a", name="w2a_t")
            w2b_t = wpool.tile([P, NUM_EXPERTS, KS, D_MODEL], f8, tag="w2b", name="w2b_t")

            # head of DMA stream: segment-0 L1 weights + first x block, pieced
            # so the first matmuls start ASAP; then the rest of the weights
            xt0 = xpool.tile([P, KD, C_BLK], f8, tag="xt0", name="xt0_0")
            xt1 = xpool.tile([P, KD, C_BLK], f8, tag="xt1", name="xt1_0")
            xtd = xpool.tile([P, KD, C_BLK], f8, tag="xtd", name="xtd_0")
            nc.sync.dma_start(w1a_t[:, 0, 0:1], w1a[:, 0, 0:1])
            for k0 in range(0, KD, 2):
                nc.sync.dma_start(xt0[:, k0 : k0 + 2], xs0[:, k0 : k0 + 2, 0])
            for k0 in range(0, KD, 2):
                nc.sync.dma_start(xt1[:, k0 : k0 + 2], xs1[:, k0 : k0 + 2, 0])
            for k0 in range(0, KD, 2):
                eng = nc.vector
                eng.tensor_scalar_mul(
                    xtd[:, k0 : k0 + 2], xt0[:, k0 : k0 + 2], 1.0 / SC
                )
            nc.sync.dma_start(w1b_t[:, 0, 0:1], w1b[:, 0, 0:1])
            for j in range(1, KS):
                nc.sync.dma_start(w1a_t[:, 0, j : j + 1], w1a[:, 0, j : j + 1])
                nc.sync.dma_start(w1b_t[:, 0, j : j + 1], w1b[:, 0, j : j + 1])

            xts = {0: (xt0, xt1, xtd)}
            # block 1's x next (L1(1) runs right after L1(0) in the pipeline),
            # then segment 0's layer-2 weights
            if NB > 1:
                nxt0 = xpool.tile([P, KD, C_BLK], f8, tag="xt0", name="xt0_1")
                nxt1 = xpool.tile([P, KD, C_BLK], f8, tag="xt1", name="xt1_1")
                nxtd = xpool.tile([P, KD, C_BLK], f8, tag="xtd", name="xtd_1")
                nc.sync.dma_start(nxt0[:], xs0[:, :, 1])
                nc.sync.dma_start(nxt1[:], xs1[:, :, 1])
                for k0 in range(0, KD, 2):
                    nc.vector.tensor_scalar_mul(
                        nxtd[:, k0 : k0 + 2], nxt0[:, k0 : k0 + 2], 1.0 / SC
                    )
                xts[1] = (nxt0, nxt1, nxtd)
            nc.sync.dma_start(w2a_t[:, 0], w2a[:, 0])
            nc.sync.dma_start(w2b_t[:, 0], w2b[:, 0])
            hts = {}

            def emit_l1(s, off, w, bidx):
                xt0, xt1, xtd = xts.pop(bidx)
                h0 = hpool.tile([P, KS, C_BLK], f8, tag="h0", name=f"h0_{bidx}")
                h1 = hpool.tile([P, KS, C_BLK], f8, tag="h1", name=f"h1_{bidx}")
                h0d = hpool.tile([P, KS, C_BLK], f8, tag="h0d", name=f"h0d_{bidx}")
                hts[bidx] = (h0, h1, h0d)
                for fc in range(KS):
                    pm = pmp.tile([P, C_BLK], f32, tag="pm", name=f"pm_{bidx}_{fc}")
                    for kp in range(KD // 2):
                        nc.tensor.matmul(
                            pm[:, :w], lhsT=w1a_t[:, s, fc, 2 * kp : 2 * kp + 2],
                            rhs=xt0[:, 2 * kp : 2 * kp + 2, :w],
                            start=(kp == 0), stop=False, perf_mode=DR,
                        )
                    for kp in range(KD // 2):
                        nc.tensor.matmul(
                            pm[:, :w], lhsT=w1a_t[:, s, fc, 2 * kp : 2 * kp + 2],
                            rhs=xt1[:, 2 * kp : 2 * kp + 2, :w],
                            start=False, stop=False, perf_mode=DR,
                        )
                    for kp in range(KD // 2):
                        nc.tensor.matmul(
                            pm[:, :w], lhsT=w1b_t[:, s, fc, 2 * kp : 2 * kp + 2],
                            rhs=xtd[:, 2 * kp : 2 * kp + 2, :w],
                            start=False, stop=(kp == KD // 2 - 1), perf_mode=DR,
                        )
                    nc.scalar.activation(h0[:, fc, :w], pm[:, :w], Relu)
                    nc.vector.scalar_tensor_tensor(
                        h1[:, fc, :w], pm[:, :w], 0.0, h0[:, fc, :w],
                        Alu.max, Alu.subtract,
                    )
                    nc.gpsimd.tensor_scalar_mul(h0d[:, fc, :w], h0[:, fc, :w], 1.0 / SC)
                # prefetch x two blocks ahead (the pipeline runs L1 a block early)
                pf = bidx + 2
                if pf < NB and pf not in xts:
                    nxt0 = xpool.tile([P, KD, C_BLK], f8, tag="xt0", name=f"xt0_{pf}")
                    nxt1 = xpool.tile([P, KD, C_BLK], f8, tag="xt1", name=f"xt1_{pf}")
                    nxtd = xpool.tile([P, KD, C_BLK], f8, tag="xtd", name=f"xtd_{pf}")
                    nc.sync.dma_start(nxt0[:], xs0[:, :, pf])
                    nc.sync.dma_start(nxt1[:], xs1[:, :, pf])
                    for k0 in range(0, KD, 2):
                        nc.vector.tensor_scalar_mul(
                            nxtd[:, k0 : k0 + 2], nxt0[:, k0 : k0 + 2], 1.0 / SC
                        )
                    xts[pf] = (nxt0, nxt1, nxtd)
                # trickle weights behind this block's x prefetch: early blocks
                # backfill seg 1-2, later the stream stays one segment ahead
                if bidx == 1:
                    nc.sync.dma_start(w1a_t[:, 1], w1a[:, 1])
                    nc.sync.dma_start(w1b_t[:, 1], w1b[:, 1])
                elif bidx == 2:
                    nc.sync.dma_start(w2a_t[:, 1], w2a[:, 1])
                    nc.sync.dma_start(w2b_t[:, 1], w2b[:, 1])
                elif bidx == 3:
                    nc.sync.dma_start(w1a_t[:, 2], w1a[:, 2])
                    nc.sync.dma_start(w1b_t[:, 2], w1b[:, 2])
                if bidx > 0 and sched[bidx - 1][0] == s - 1:
                    if s + 1 < NUM_EXPERTS:
                        nc.sync.dma_start(w2a_t[:, s + 1], w2a[:, s + 1])
                        nc.sync.dma_start(w2b_t[:, s + 1], w2b[:, s + 1])
                    if s + 2 < NUM_EXPERTS:
                        nc.sync.dma_start(w1a_t[:, s + 2], w1a[:, s + 2])
                        nc.sync.dma_start(w1b_t[:, s + 2], w1b[:, s + 2])

            def emit_l2(s, off, w, bidx):
                h0, h1, h0d = hts.pop(bidx)
                for dt in range(KD):
                    pym = pymp.tile([P, C_BLK], f32, tag="pym", name=f"pym_{bidx}_{dt}")
                    dsl = slice(dt * P, (dt + 1) * P)
                    for kp in range(KS // 2):
                        nc.tensor.matmul(
                            pym[:, :w], lhsT=w2a_t[:, s, 2 * kp : 2 * kp + 2, dsl],
                            rhs=h0[:, 2 * kp : 2 * kp + 2, :w],
                            start=(kp == 0), stop=False, perf_mode=DR,
                        )
                    for kp in range(KS // 2):
                        nc.tensor.matmul(
                            pym[:, :w], lhsT=w2a_t[:, s, 2 * kp : 2 * kp + 2, dsl],
                            rhs=h1[:, 2 * kp : 2 * kp + 2, :w],
                            start=False, stop=False, perf_mode=DR,
                        )
                    for kp in range(KS // 2):
                        nc.tensor.matmul(
                            pym[:, :w], lhsT=w2b_t[:, s, 2 * kp : 2 * kp + 2, dsl],
                            rhs=h0d[:, 2 * kp : 2 * kp + 2, :w],
                            start=False, stop=(kp == KS // 2 - 1), perf_mode=DR,
                        )
                    yt = ypool.tile([P, C_BLK], bf16, tag="yt", name=f"yt_{bidx}_{dt}")
                    nc.scalar.copy(yt[:, :w], pym[:, :w])
                    nc.sync.dma_start(yT[:, dt, off : off + w], yt[:, :w])

            # depth-2 software pipeline: L1 runs one block ahead of L2, so
            # each L2's weight/copy dependencies resolve behind L1 compute
            emit_l1(*sched[0])
            for i in range(len(sched)):
                if i + 1 < NB:
                    emit_l1(*sched[i + 1])
                emit_l2(*sched[i])

    nc.compile()
    return nc


def route_tokens(xf: np.ndarray, gate_w: np.ndarray):
    logits = xf @ gate_w.astype(np.float32).T
    top2 = np.argsort(-logits, axis=-1, kind="stable")[:, :TOP_K]
    tv = np.take_along_axis(logits, top2, axis=-1)
    tv = tv - tv.max(axis=-1, keepdims=True)
    ex = np.exp(tv)
    gates = ex / ex.sum(axis=-1, keepdims=True)
    rows, weights = [], []
    for e in range(NUM_EXPERTS):
        r, kpos = np.nonzero(top2 == e)
        rows.append(r)
        weights.append(gates[r, kpos].astype(np.float32))
    return rows, weights


def _fp8_pair(a: np.ndarray, scale: float = 1.0):
    a0 = a.astype(F8)
    a1 = ((a - a0.astype(np.float32)) * scale).astype(F8)
    return a0, a1


def make_inputs(xf, w1, w2, rows, weights):
    seg_lens = tuple(len(r) for r in rows)
    C = sum(seg_lens)
    sched = _seg_blocks(seg_lens)
    NB = len(sched)

    Xg = np.empty((C, D_MODEL), np.float32)
    o = 0
    for e in range(NUM_EXPERTS):
        n = len(rows[e])
        Xg[o : o + n] = xf[rows[e]] * weights[e][:, None]
        o += n
    X0, X1 = _fp8_pair(Xg)

    def xlay(X):
        xt = X.T.reshape(KD, P, C).transpose(1, 0, 2)  # [P, KD, C]
        out = np.zeros((P, KD, NB, C_BLK), X.dtype)
        for s, off, w, bidx in sched:
            out[:, :, bidx, :w] = xt[:, :, off : off + w]
        return out

    xs0, xs1 = xlay(X0), xlay(X1)

    # full-expert weight layouts, then slice per core
    w1a_full, w1b_full, w2a_full, w2b_full = [], [], [], []
    for e in range(NUM_EXPERTS):
        A, B = _fp8_pair(w1[e].astype(np.float32), SC)
        w1a_full.append(A.reshape(KF := D_FF // P, P, KD, P).transpose(3, 0, 2, 1))
        w1b_full.append(B.reshape(KF, P, KD, P).transpose(3, 0, 2, 1))
        A2, B2 = _fp8_pair(w2[e].astype(np.float32), SC)
        w2a_full.append(A2.T.reshape(KF, P, D_MODEL).transpose(1, 0, 2))
        w2b_full.append(B2.T.reshape(KF, P, D_MODEL).transpose(1, 0, 2))

    in_maps = []
    for i in range(NUM_EXPERTS):  # core i -> F slice i
        sl = slice(KS * i, KS * (i + 1))
        in_maps.append(
            {
                "xs0": xs0,
                "xs1": xs1,
                "w1a": np.ascontiguousarray(
                    np.stack([w1a_full[e][:, sl] for e in range(NUM_EXPERTS)], axis=1)
                ),
                "w1b": np.ascontiguousarray(
                    np.stack([w1b_full[e][:, sl] for e in range(NUM_EXPERTS)], axis=1)
                ),
                "w2a": np.ascontiguousarray(
                    np.stack([w2a_full[e][:, sl] for e in range(NUM_EXPERTS)], axis=1)
                ),
                "w2b": np.ascontiguousarray(
                    np.stack([w2b_full[e][:, sl] for e in range(NUM_EXPERTS)], axis=1)
                ),
            }
        )
    return in_maps, seg_lens


def kernel(x, gate_w, w1, w2):
    from concourse.bass_utils import run_bass_kernel_spmd

    x = np.asarray(x)
    B, S, D = x.shape
    xf = x.reshape(-1, D).astype(np.float32)
    rows, weights = route_tokens(xf, np.asarray(gate_w))
    in_maps, seg_lens = make_inputs(xf, np.asarray(w1), np.asarray(w2), rows, weights)

    nc = _NC_CACHE.get(seg_lens)
    if nc is None:
        nc = _NC_CACHE[seg_lens] = build_moe_nc2(seg_lens)
    res = run_bass_kernel_spmd(nc, in_maps, core_ids=list(range(NUM_EXPERTS)))

    C = sum(seg_lens)
    ysum = np.zeros((C, D_MODEL), np.float32)
    for c in range(NUM_EXPERTS):
        yT = res.results[c]["yT"].astype(np.float32)  # [P, KD, C]
        ysum += yT.transpose(2, 1, 0).reshape(C, D_MODEL)
    out = np.zeros((B * S, D), np.float32)
    o = 0
    for e in range(NUM_EXPERTS):
        n = len(rows[e])
        out[rows[e]] += ysum[o : o + n]
        o += n
    return out.reshape(B, S, D)
